# revision 75
# baseline (speedup 1.0000x reference)
"""Self-contained Trainium2 Bass kernel for nn_MultiHeadAttention_65060164600355.

Full inputs in, full output out. Sharding: 8 cores = (batch b, query-row half),
core c -> b = c//2, query rows [1024*(c%2), 1024*(c%2)+1024). Each core
duplicates the K/V projections for its batch (no cross-core communication;
output assembly is pure concatenation).

v2: all matmul I/O in bf16 (fp32 PSUM accumulation), K^T / V / Q^T fully
SBUF-resident (no DRAM bounce), V projected directly into [k, dh] layout,
1024-wide exp with the 1/sqrt(D) scale folded in, explicit engine routing
(ACT = exp only), interleaved emission so attention overlaps projections.
"""

import numpy as np

# ---------------------------------------------------------------------------
# Workarounds for this container's walrus build (max ONE sem-wait command per
# instruction; TileContext's end-of-kernel Drain must carry none).
# ---------------------------------------------------------------------------
import concourse.tile as tile_mod
from concourse.vector_clock import ScopedClock, VectorClock


def _drain_and_barrier(self, tick_clock, wait_clock):
    nc = self.nc
    vc = tick_clock.global_clock
    n = len(vc)
    for i in range(n):
        t = vc[i]
        if t > 0:
            vec = [0] * n
            vec[i] = t
            nop_inst = nc.sync.nop(nofuse=True, hint=f"tile_drain_wait_{i}")
            wait_clock.add_sem_waits(
                nop_inst.ins, ScopedClock({None: VectorClock(vec)})
            )
    nc.sync.drain()
    nc.all_engine_barrier()
    assert self.sems is not None
    popped = nc._tile_sem_poison_stack.pop()
    assert popped is self._sem_poison
    nc.clear_and_free_semaphores(list(self.sems.allocated().values()))
    nc.all_engine_barrier()


tile_mod.TileContext._drain_and_barrier = _drain_and_barrier

import concourse.mybir as _mybir


def legalize_waits(nc, max_waits=1):
    """This container's walrus accepts at most one sem-wait command per
    instruction. Hoist excess waits onto NoOps inserted just before the
    instruction in its basic block (same engine => same program order)."""
    ctr = 0
    for f in nc.m.functions:
        for bb in f.blocks:
            out = []
            changed = False
            for inst in bb.instructions:
                si = inst.sync_info
                if si is not None and si.on_wait and len(si.on_wait) > max_waits:
                    waits = list(si.on_wait)
                    for w in waits[:-max_waits]:
                        nop = _mybir.InstNoOp(name=f"waitfix_nop_{ctr}", ins=[], outs=[])
                        ctr += 1
                        nop.engine = inst.engine
                        nop.sync_info = _mybir.SyncInfo(on_wait=[w], on_update=[])
                        out.append(nop)
                    inst.sync_info = _mybir.SyncInfo(
                        on_wait=waits[-max_waits:], on_update=list(si.on_update)
                    )
                    changed = True
                out.append(inst)
            if changed:
                bb.instructions = out
    return ctr


# ---------------------------------------------------------------------------
# Kernel builder
# ---------------------------------------------------------------------------

from contextlib import ExitStack

import concourse.bass as bass
import concourse.mybir as mybir
import concourse.tile as tile
from concourse.masks import make_identity

F32 = mybir.dt.float32
F32R = mybir.dt.float32r
BF16 = mybir.dt.bfloat16
EXP = mybir.ActivationFunctionType.Exp


def build(S=2048, SQ=1024, D=1024, H=16):
    DH = 64
    NPAIR = H // 2        # head pairs; pair i covers dout cols i*128..i*128+127
    DT = D // 128         # din tiles
    KT = S // 128         # k tiles of 128
    KC = S // 512         # k chunks of 512
    QC = SQ // 512        # q chunks of 512
    QT = SQ // 128        # q row tiles
    scale = 1.0 / float(D) ** 0.5

    nc = bass.Bass()
    q_d = nc.dram_tensor("q", [SQ, D], F32, kind="ExternalInput")
    k_d = nc.dram_tensor("k", [S, D], F32, kind="ExternalInput")
    v_d = nc.dram_tensor("v", [S, D], F32, kind="ExternalInput")
    w_d = {n: nc.dram_tensor(n, [D, D], F32, kind="ExternalInput")
           for n in ("wq", "wk", "wv", "wo")}
    out_d = nc.dram_tensor("out", [SQ, D], F32, kind="ExternalOutput")

    with tile.TileContext(nc) as tc, ExitStack() as ctx:
        # ---------------- persistent SBUF ----------------
        singles = ctx.enter_context(tc.tile_pool(name="singles", bufs=1))
        ident = singles.tile([128, 128], F32)
        make_identity(nc, ident)
        identr = singles.tile([128, 128], F32R)
        nc.vector.tensor_copy(identr[:], ident[:])
        ones32 = singles.tile([128, 1], F32)
        nc.vector.memset(ones32[:], 1.0)
        ones_r = singles.tile([1, 64], F32R)
        nc.vector.tensor_copy(
            ones_r[:], ones32[0:1, None, :].to_broadcast((1, 64, 1)))

        resid = ctx.enter_context(tc.tile_pool(name="resid", bufs=1))
        kT = resid.tile([128, NPAIR, S], BF16, tag="kT")       # [dh-of-pair, pair, k]
        vN = resid.tile([128, KT, NPAIR, 130], BF16, tag="vN")  # [k, ktile, pair, dh+ones]
        qt = resid.tile([128, NPAIR, SQ], BF16, tag="qt")      # [dh-of-pair, pair, q]
        ctxT = resid.tile([128, NPAIR, SQ], BF16, tag="ctxT")  # [dh-of-pair, pair, q]
        # ones columns for the denominator rows of ctx matmuls
        nc.vector.memset(vN[:, :, :, 64:65], 1.0)
        nc.vector.memset(vN[:, :, :, 129:130], 1.0)

        wpool = ctx.enter_context(tc.tile_pool(name="wpool", bufs=2))
        wstage = ctx.enter_context(tc.tile_pool(name="wstage", bufs=2))
        xtv_pool = ctx.enter_context(tc.tile_pool(name="xtv", bufs=1))
        xtv = xtv_pool.tile([128, DT, S], BF16)                 # resident V^T
        xtq1_pool = ctx.enter_context(tc.tile_pool(name="xtq1", bufs=1))
        xtq1 = xtq1_pool.tile([128, DT, 512], BF16)             # Q^T chunk 1

        # PSUM pool that must live through projections AND attention
        psProj = ctx.enter_context(tc.tile_pool(name="psProj", bufs=2, space="PSUM"))

        def load_w(name, dma_eng=None):
            """Stage one [D, D] weight: 8 chunk DMAs + casts split DVE/Pool."""
            dma_eng = dma_eng or nc.sync
            wb = wpool.tile([128, DT, D], BF16, tag="wb", name=f"wb_{name}")
            for dt in range(DT):
                wf = wstage.tile([128, D], F32, tag="wf", name=f"wf_{name}{dt}")
                dma_eng.dma_start(wf[:], w_d[name][dt * 128:(dt + 1) * 128, :])
                eng = nc.vector if dt % 2 == 0 else nc.gpsimd
                eng.tensor_copy(wb[:, dt, :], wf[:])
            return wb

        def proj_v_pair(wv, i, t):
            """vN[:, t, i, dh-cols]: one pair, one k tile ([128,128])."""
            ps = psProj.tile([128, 512], F32, tag="pj", name="psv")
            for dt in range(DT):
                nc.tensor.matmul(
                    ps[:, 0:128],
                    xtv[:, dt, t * 128:(t + 1) * 128],
                    wv[:, dt, i * 128:(i + 1) * 128],
                    start=(dt == 0), stop=(dt == DT - 1))
            # strided store: [2 heads x 64] around the ones columns
            base = vN[:, t, i, 0:64]
            dst = bass.AP(
                tensor=base.tensor, offset=base.offset,
                ap=[list(base.ap[0]), [65, 2], list(base.ap[1])])
            nc.vector.tensor_copy(dst, ps[:, 0:128])

        # ---------------- phase 1: loads, casts, transposes, K/Q proj ------
        with ExitStack() as p1:
            xstage = p1.enter_context(tc.tile_pool(name="xstage", bufs=3))
            xt_pool = p1.enter_context(tc.tile_pool(name="xt", bufs=2))
            psT = p1.enter_context(tc.tile_pool(name="psT", bufs=4, space="PSUM"))

            wk = load_w("wk", dma_eng=nc.gpsimd)

            def transpose_chunk(x_dram, s0, dst, dst_off, dma_eng=None):
                """dst[:, dt, dst_off:dst_off+512] = x[s0:s0+512, :].T (bf16)"""
                dma_eng = dma_eng or nc.sync
                for st in range(4):
                    xs = xstage.tile([128, D], F32R, tag="xs")
                    dma_eng.dma_start(
                        xs[:],
                        x_dram[s0 + st * 128:s0 + (st + 1) * 128, :].bitcast(F32R))
                    for dg in range(2):           # groups of 4 din blocks
                        pt = psT.tile([128, 4, 128], F32R, tag="pt")
                        for dd in range(4):
                            dt = dg * 4 + dd
                            nc.tensor.transpose(
                                pt[:, dd, :],
                                xs[:, dt * 128:(dt + 1) * 128],
                                identr[:])
                        nc.scalar.copy(
                            dst[:, dg * 4:(dg + 1) * 4,
                                dst_off + st * 128:dst_off + (st + 1) * 128],
                            pt[:])

            def proj_chunk(wb, xt, c, dst):
                """dst[:, p, c*512:(c+1)*512] = (x W)^T for all pairs."""
                for p in range(NPAIR):
                    ps = psProj.tile([128, 512], F32, tag="pj")
                    for dt in range(DT):
                        nc.tensor.matmul(
                            ps[:], wb[:, dt, p * 128:(p + 1) * 128], xt[:, dt, :],
                            start=(dt == 0), stop=(dt == DT - 1))
                    nc.scalar.copy(dst[:, p, c * 512:(c + 1) * 512], ps[:])

            # K and V interleaved, chunk-major; V projection for pair 0 rides
            # along so the attention rounds only carry pairs 1-7.
            wv = None
            for c in range(KC):
                xt = xt_pool.tile([128, DT, 512], BF16, tag="xt")
                transpose_chunk(k_d, c * 512, xt, 0)
                proj_chunk(wk, xt, c, kT)
                transpose_chunk(v_d, c * 512, xtv, c * 512)
                if c == 0:
                    wv = load_w("wv")
                if c >= 1:
                    for t in range(4 * (c - 1), 4 * c):
                        proj_v_pair(wv, 0, t)

            # Q chunk 0 (wq reuses wk's slot; the V-proj tail covers the
            # cast latency)
            wq = load_w("wq")
            xtq0 = xt_pool.tile([128, DT, 512], BF16, tag="xt")
            transpose_chunk(q_d, 0, xtq0, 0, dma_eng=nc.gpsimd)
            for t in range(12, KT):
                proj_v_pair(wv, 0, t)
            proj_chunk(wq, xtq0, 0, qt)

            # Q chunk 1: transpose now (psT dies with phase 1); project later,
            # spread across the attention stream
            transpose_chunk(q_d, 512, xtq1, 0, dma_eng=nc.gpsimd)

        # ---------------- phase 2: attention + V proj + out proj -----------
        with ExitStack() as p2:
            e_pool = p2.enter_context(tc.tile_pool(name="e", bufs=6))
            nrm_pool = p2.enter_context(tc.tile_pool(name="nrm", bufs=2))
            outb_pool = p2.enter_context(tc.tile_pool(name="outb", bufs=2))
            psS = p2.enter_context(tc.tile_pool(name="psS", bufs=2, space="PSUM"))
            psC = p2.enter_context(tc.tile_pool(name="psC", bufs=1, space="PSUM"))

            def q1_fillers(p):
                """qt[:, p, 512:1024] as two ~0.85us filler closures; the
                accumulation group interleaves with other-bank matmuls."""
                cell = {}

                def part0():
                    cell["ps"] = psProj.tile([128, 512], F32, tag="pj",
                                             name="psq")
                    for dt in range(4):
                        nc.tensor.matmul(
                            cell["ps"][:], wq[:, dt, p * 128:(p + 1) * 128],
                            xtq1[:, dt, :], start=(dt == 0), stop=False,
                            skip_group_check=True)

                def part1():
                    ps = cell["ps"]
                    for dt in range(4, DT):
                        nc.tensor.matmul(
                            ps[:], wq[:, dt, p * 128:(p + 1) * 128],
                            xtq1[:, dt, :], start=False, stop=(dt == DT - 1),
                            skip_group_check=True)
                    nc.vector.tensor_copy(qt[:, p, 512:1024], ps[:])

                return [part0, part1]

            def out_proj_fillers(qtile, wo):
                """Out projection for one q tile as four filler closures."""
                cell = {}

                def mm(dc, p0, p1, start, stop):
                    for p in range(p0, p1):
                        nc.tensor.matmul(
                            cell["ps"][:],
                            ctxT[:, p, qtile * 128:(qtile + 1) * 128],
                            wo[:, p, dc * 512:(dc + 1) * 512],
                            start=start and (p == p0), stop=stop and (p == p1 - 1),
                            skip_group_check=True)

                def c0():
                    cell["ob"] = outb_pool.tile([128, D], F32, tag="ob", name="ob")
                    cell["ps"] = psProj.tile([128, 512], F32, tag="pj",
                                             name="pso")
                    mm(0, 0, 4, True, False)

                def c1():
                    mm(0, 4, NPAIR, False, True)
                    nc.vector.tensor_copy(cell["ob"][:, 0:512], cell["ps"][:])

                def c2():
                    cell["ps"] = psProj.tile([128, 512], F32, tag="pj",
                                             name="pso")
                    mm(1, 0, 4, True, False)

                def c3():
                    mm(1, 4, NPAIR, False, True)
                    nc.vector.tensor_copy(cell["ob"][:, 512:1024], cell["ps"][:])
                    nc.sync.dma_start(
                        out_d[qtile * 128:(qtile + 1) * 128, :], cell["ob"][:])

                return [c0, c1, c2, c3]

            def out_proj_qtile(qtile, wo):
                for f in out_proj_fillers(qtile, wo):
                    f()

            def score_tile(i, c, t):
                """scores + exp for (pair i, qchunk c, ktile t); returns e."""
                sc = psS.tile([128, 2, 512], F32, tag="sc")
                for j in range(2):
                    nc.tensor.matmul(
                        sc[:, j, :],
                        kT[j * 64:(j + 1) * 64, i, t * 128:(t + 1) * 128],
                        qt[j * 64:(j + 1) * 64, i, c * 512:(c + 1) * 512],
                        start=True, stop=True)
                e = e_pool.tile([128, 2, 512], BF16, tag="e")
                nc.scalar.activation(e[:], sc[:], EXP, scale=scale)
                return e

            def ctx_tile(i, t, pcs, e):
                for j in range(2):
                    nc.tensor.matmul(
                        pcs[j][:65, :], vN[:, t, i, j * 65:(j + 1) * 65],
                        e[:, j, :], start=(t == 0), stop=(t == KT - 1))

            def attn_norm(i, c, pcs):
                """normalize ctx rows by the denominator row, write ctxT.

                The per-q reciprocal (a row) is broadcast across 64 partitions
                with a rank-1 PE matmul: ones[1,64]^T @ rcp[1,512]."""
                for j in range(2):
                    nrm = nrm_pool.tile([65, 512], F32R, tag="nrm")
                    nc.vector.tensor_copy(nrm[:], pcs[j][:65, :])
                    rcp = nrm_pool.tile([1, 512], F32R, tag="rcp")
                    with nc.allow_low_precision(reason="f32r is bit-identical f32"):
                        nc.vector.reciprocal(rcp[:], nrm[64:65, :])
                    bc = psProj.tile([128, 512], F32, tag="pj", name="bc")
                    nc.tensor.matmul(bc[0:64, :], ones_r[:], rcp[:],
                                     start=True, stop=True)
                    nc.vector.tensor_tensor(
                        ctxT[j * 64:(j + 1) * 64, i, c * 512:(c + 1) * 512],
                        nrm[:64, :], bc[0:64, :], mybir.AluOpType.mult)

            def attn_block(i, c, fillers=()):
                """One (pair, qchunk): 16 k tiles, software-pipelined so the
                scores matmul for tile t+1 issues before ctx of tile t (hiding
                the exp latency), with filler PE work in the wait slots.
                Returns a closure that emits this block's normalization, to be
                run as a filler inside the NEXT block (so the reciprocal chain
                doesn't head-block the PE stream)."""
                fillers = list(fillers)
                pcs = [psC.tile([128, 512], F32, tag=f"ctx{j}",
                                name=f"pcs{j}") for j in range(2)]
                e_prev = score_tile(i, c, 0)
                for t in range(1, KT):
                    if fillers:
                        fillers.pop(0)()
                    e = score_tile(i, c, t)
                    ctx_tile(i, t - 1, pcs, e_prev)
                    e_prev = e
                for f in fillers:
                    f()
                ctx_tile(i, KT - 1, pcs, e_prev)
                attn_norm(i, c, pcs)

            # qchunk-0 round: pair i's block carries pair i+1's V projection;
            # the last block also projects Q-chunk-1 for pair 0
            for i in range(NPAIR):
                fillers = []
                if i + 1 < NPAIR:
                    fillers += [
                        (lambda t=t, p=i + 1: proj_v_pair(wv, p, t))
                        for t in range(KT)]
                else:
                    fillers += q1_fillers(0)
                attn_block(i, 0, fillers)
                if i == 3:
                    wo = load_w("wo")

            # qchunk-1 round: carries the remaining Q-chunk-1 pieces and the
            # first half of the out projection, split into fine filler parts
            for i in range(NPAIR):
                fillers = []
                if i + 1 < NPAIR:
                    fillers += q1_fillers(i + 1)
                if i % 2 == 1:
                    fillers += out_proj_fillers(i // 2, wo)
                attn_block(i, 1, fillers)

            # ---------------- out projection (second half) ----------------
            for qtile in range(QT // 2, QT):
                out_proj_qtile(qtile, wo)

    return nc


# ---------------------------------------------------------------------------
# Host wrapper
# ---------------------------------------------------------------------------
from concourse.bass_utils import run_bass_kernel_spmd

B, S, D, H = 4, 2048, 1024, 16
SQ = S // 2
_NC = None
PROFILE = False
LAST_EXEC_NS = None


def _get_nc():
    global _NC
    if _NC is None:
        _NC = build(S=S, SQ=SQ, D=D, H=H)
        legalize_waits(_NC)
    return _NC


def kernel(queries, keys, values, Wq, Wk, Wv, Wo):
    global LAST_EXEC_NS
    nc = _get_nc()
    in_maps = []
    for c in range(8):
        b, half = c // 2, c % 2
        in_maps.append({
            "q": np.ascontiguousarray(queries[b, half * SQ:(half + 1) * SQ, :]),
            "k": np.ascontiguousarray(keys[b]),
            "v": np.ascontiguousarray(values[b]),
            "wq": np.asarray(Wq), "wk": np.asarray(Wk),
            "wv": np.asarray(Wv), "wo": np.asarray(Wo),
        })
    res = run_bass_kernel_spmd(nc, in_maps, list(range(8)), trace=PROFILE)
    LAST_EXEC_NS = res.exec_time_ns
    out = np.empty((B, S, D), np.float32)
    for c in range(8):
        out[c // 2, (c % 2) * SQ:(c % 2 + 1) * SQ, :] = res.results[c]["out"]
    return out
